# revision 3
# baseline (speedup 1.0000x reference)
"""Trainium2 Bass kernel for nn_DiscreteTimeS4.

Model (reference):
    x_proj = relu(x_seq @ W_in^T + b_in)                  # [B, T, P]
    h_t = a * h_{t-1} + x_proj_t @ B ;  y_t = h_t @ C     # diagonal SSM scan
    out = y @ W_out^T + b_out                             # [B, T, OUT]

Key transform: |a| <= sqrt(2/H) ~ 0.09, so a^k decays below the 2e-2
correctness gate within a couple of steps.  The scan is a short causal
convolution over time with W_out folded in:
    out_t = sum_k x_proj_{t-k} @ F_k,   F_k = B @ diag(a^k) @ C @ W_out^T

Device pipeline (per core, batch rows processed in PAIRS so that the
full 128-partition dim is used with no duplicated input DMA):
    stage 1: for row pair rp, chunk c: two row-group-packed matmuls
             (K=64 contraction each) -> PSUM [128, 2*512] (both rows),
             then one fused bias+relu op (DVE or ACT) -> xp tile f16.
    stage 2: for chunk pair: per chunk, K taps x 2 col-groups of
             F_k^T @ xp(shift k), PSUM-accumulated -> [128(=2 rows x
             64 outs), 1024]; lag shifts are SBUF column offsets.
    out:     one copy op (DVE/ACT) PSUM -> SBUF f16, then one
             contiguous DMA -> out dram [BL*OUT, T] f16.
Host side: fold F_k in fp64, pre-transpose x to [BL*IN, T] f16,
post-transpose out back to [B, T, OUT] fp32 (b_out is zero here but
added on host if nonzero).

Sharding: data-parallel over batch, 8 NeuronCores, B=64 -> 8 per core.
"""

import os
import sys

for _p in ("/opt/trn_rl_repo", "/root/.axon_site/_ro/trn_rl_repo"):
    if os.path.isdir(_p) and _p not in sys.path:
        sys.path.append(_p)

import numpy as np

import concourse.bacc as bacc
import concourse.mybir as mybir
from concourse.bass_utils import run_bass_kernel_spmd
from concourse.tile import TileContext

BATCH, T, IN, P, H, OUT = 64, 2048, 64, 128, 256, 64
NCORES = 8
BL = BATCH // NCORES          # batches per core
RP = BL // 2                  # row pairs per core
CHUNK = 512                   # time chunk (one fp32 PSUM bank)
NCHUNK = T // CHUNK
NP2 = NCHUNK // 2             # chunk pairs per row

F32 = mybir.dt.float32
F16 = mybir.dt.float16

_programs = {}                # (n_lags, reps) -> finalized Bacc program


def _build(n_lags: int, reps: int = 1):
    """Build the per-core Bass program for a fixed lag count.

    reps > 1 wraps the whole computation in an on-device loop executing
    it `reps` times — used only for benchmarking (amortizes the axon
    dispatch overhead, which dwarfs the kernel itself).
    """
    import contextlib

    nc = bacc.Bacc("TRN2", target_bir_lowering=False, num_devices=NCORES)

    x = nc.declare_dram_parameter("x", [BL * IN, T], F16, isOutput=False)
    wfold = nc.declare_dram_parameter("wfold", [n_lags, P, OUT], F16,
                                      isOutput=False)
    # W_in^T duplicated into both partition halves for row-group packing
    w_inT = nc.declare_dram_parameter("w_inT", [2 * IN, P], F16, isOutput=False)
    b_in = nc.declare_dram_parameter("b_in", [P, 1], F32, isOutput=False)
    out = nc.declare_dram_parameter("out", [BL * OUT, T], F16, isOutput=True)

    with TileContext(nc) as tc:
        with (
            tc.tile_pool(name="wpool", bufs=1) as wpool,
            tc.tile_pool(name="xin", bufs=2) as xin_pool,
            tc.tile_pool(name="xproj", bufs=8) as xp_pool,
            tc.tile_pool(name="obuf", bufs=3) as ob_pool,
            tc.tile_pool(name="ps1", bufs=2, space="PSUM") as ps1_pool,
            tc.tile_pool(name="pso", bufs=2, space="PSUM") as pso_pool,
        ):
            # ---- load weights once (already fp16 from host) ----
            fk = wpool.tile([P, n_lags * OUT], F16)
            nc.sync.dma_start(out=fk[:].rearrange("p (k o) -> p k o",
                                                  k=n_lags),
                              in_=wfold[:].rearrange("k p o -> p k o"))
            wi = wpool.tile([2 * IN, P], F16)
            nc.sync.dma_start(out=wi[:], in_=w_inT[:])
            bi = wpool.tile([P, 1], F32)
            nc.sync.dma_start(out=bi[:], in_=b_in[:])

            rep_ctx = (
                tc.For_i(
                    0, reps, 1,
                    hint_engines=(
                        mybir.EngineType.PE,
                        mybir.EngineType.DVE,
                        mybir.EngineType.Activation,
                        mybir.EngineType.SP,
                        mybir.EngineType.Pool,
                    ),
                )
                if reps > 1
                else contextlib.nullcontext()
            )
            with rep_ctx:
                _emit_body(nc, tc, n_lags, x, out, fk, wi, bi,
                           xin_pool, xp_pool, ob_pool, ps1_pool, pso_pool)

    nc.finalize()
    return nc


def _emit_body(nc, tc, n_lags, x, out, fk, wi, bi,
               xin_pool, xp_pool, ob_pool, ps1_pool, pso_pool):
    PAD = n_lags - 1
    XW = PAD + CHUNK            # xp columns per row half

    # Elementwise engine balancing: DVE ~245 G elem/s, ACT ~154 G elem/s.
    # All big elementwise ops are [128, 1024] fp32-src; alternate with a
    # weighted round-robin so both engines finish together.
    load = {"v": 0.0, "a": 0.0}
    COST = {"v": 0.74, "a": 1.15}

    def pick_engine():
        e = "v" if load["v"] + COST["v"] <= load["a"] + COST["a"] else "a"
        load[e] += COST[e]
        return e

    def load_x(rp):
        xTr = xin_pool.tile([2 * IN, T], F16, tag="xTr")
        nc.sync.dma_start(out=xTr[:], in_=x[rp * 128:(rp + 1) * 128, :])
        return xTr

    def stage1(rp, c, xTr, xp_prev):
        """Input projection + bias-relu for chunk c of both rows of
        pair rp -> xp tile [128, 2*XW] f16 (halves = rows 2rp, 2rp+1,
        each PAD lag cols + CHUNK cols)."""
        ps1 = ps1_pool.tile([P, 2 * CHUNK], F32)
        for h in range(2):
            nc.tensor.matmul(
                ps1[:, h * CHUNK:(h + 1) * CHUNK],
                wi[h * IN:(h + 1) * IN, :],
                xTr[h * IN:(h + 1) * IN, c * CHUNK:(c + 1) * CHUNK],
                start=True, stop=True,
                tile_position=(h * IN, 0),
            )
        xp = xp_pool.tile([P, 2 * XW], F16, tag="xpp")
        xp3 = xp[:].rearrange("p (h z) -> p h z", h=2)
        if PAD > 0:
            if c == 0:
                nc.gpsimd.memset(xp3[:, :, 0:PAD], 0.0)
            else:
                xq3 = xp_prev[:].rearrange("p (h z) -> p h z", h=2)
                nc.gpsimd.tensor_copy(out=xp3[:, :, 0:PAD],
                                      in_=xq3[:, :, CHUNK:CHUNK + PAD])
        dst = xp3[:, :, PAD:PAD + CHUNK]
        src = ps1[:].rearrange("p (h t) -> p h t", h=2)
        if pick_engine() == "v":
            nc.vector.tensor_scalar(
                out=dst, in0=src, scalar1=bi[:], scalar2=0.0,
                op0=mybir.AluOpType.add, op1=mybir.AluOpType.max,
            )
        else:
            nc.scalar.activation(
                out=dst, in_=src,
                func=mybir.ActivationFunctionType.Relu, bias=bi[:],
            )
        return xp

    def stage2(rp, p, xpA, xpB):
        """Fused conv for chunk pair (2p, 2p+1) of row pair rp.
        Col-groups h = the two rows run concurrently in disjoint PE
        column groups -> pso [128 (= 2 rows x 64 outs), 1024]."""
        pso = pso_pool.tile([2 * OUT, 2 * CHUNK], F32)
        for q, xq in enumerate((xpA, xpB)):
            xq3 = xq[:].rearrange("p (h z) -> p h z", h=2)
            for k in range(n_lags):
                for h in range(2):
                    nc.tensor.matmul(
                        pso[h * OUT:(h + 1) * OUT,
                            q * CHUNK:(q + 1) * CHUNK],
                        fk[:, k * OUT:(k + 1) * OUT],
                        xq3[:, h, PAD - k:PAD - k + CHUNK],
                        start=(k == 0), stop=(k == n_lags - 1),
                        tile_position=(0, h * OUT),
                    )
        ob = ob_pool.tile([2 * OUT, 2 * CHUNK], F16)
        if pick_engine() == "v":
            nc.vector.tensor_copy(out=ob[:], in_=pso[:])
        else:
            nc.scalar.activation(out=ob[:], in_=pso[:],
                                 func=mybir.ActivationFunctionType.Copy)
        nc.sync.dma_start(
            out=out[rp * 128:(rp + 1) * 128,
                    p * 2 * CHUNK:(p + 1) * 2 * CHUNK],
            in_=ob[:],
        )

    # Software pipeline: stage2 of a chunk pair is emitted DEPTH
    # stage1-steps after its inputs are produced, so PE work stays dense
    # while bias-relu results land.
    DEPTH = 2
    s2q = []
    xTr_cur = None
    xp_prev = None
    for rp in range(RP):
        for c in range(NCHUNK):
            if c == 0:
                xTr_cur = load_x(rp)
                xp_prev = None
            xp_c = stage1(rp, c, xTr_cur, xp_prev)
            if c % 2 == 1:
                s2q.append((rp, c // 2, xp_prev, xp_c))
            xp_prev = xp_c
            while len(s2q) > DEPTH:
                stage2(*s2q.pop(0))
    while s2q:
        stage2(*s2q.pop(0))


def _n_lags(a: np.ndarray) -> int:
    amax = float(np.abs(a).max())
    if amax >= 1.0:
        return 16
    if amax <= 0.0:
        return 2
    # Correctness gate is rel_err < 2e-2 (max-normalized).  Truncating
    # at a^k < 8e-3 gives measured end-to-end error ~3.7e-3 for this
    # model (K=2), a 5x margin; the fp16 operand noise floor is ~5e-4.
    k = int(np.ceil(np.log(8e-3) / np.log(amax)))
    return max(2, min(16, k))


def _prepare(x_seq, a, B, C, W_in, b_in, W_out, b_out):
    """Host-side folding + per-core input maps."""
    n_lags = _n_lags(a)
    a64 = a.astype(np.float64)
    B64 = B.astype(np.float64)
    C64 = C.astype(np.float64)
    CW64 = C64 @ W_out.T.astype(np.float64)                # [H, OUT]
    fks = np.stack(
        [(B64 * (a64 ** k)[None, :]) @ CW64 for k in range(n_lags)]
    ).astype(np.float16)                                   # [K, P, OUT]
    wiT = W_in.T.astype(np.float16)
    shared = {
        "wfold": np.ascontiguousarray(fks),
        "w_inT": np.ascontiguousarray(np.vstack([wiT, wiT])),
        "b_in": np.ascontiguousarray(b_in.astype(np.float32).reshape(P, 1)),
    }
    xT = np.swapaxes(x_seq, 1, 2).astype(np.float16)       # [B, IN, T]
    in_maps = []
    for c in range(NCORES):
        m = dict(shared)
        m["x"] = np.ascontiguousarray(
            xT[c * BL:(c + 1) * BL].reshape(BL * IN, T))
        in_maps.append(m)
    return n_lags, in_maps


def get_program(n_lags: int, reps: int = 1):
    key = (n_lags, reps)
    if key not in _programs:
        _programs[key] = _build(n_lags, reps)
    return _programs[key]


def kernel(x_seq, a, B, C, W_in, b_in, W_out, b_out):
    n_lags, in_maps = _prepare(x_seq, a, B, C, W_in, b_in, W_out, b_out)
    nc = get_program(n_lags)
    res = run_bass_kernel_spmd(nc, in_maps, list(range(NCORES)))
    outs = []
    for c in range(NCORES):
        o = np.asarray(res.results[c]["out"])              # [BL*OUT, T] f16
        o = o.reshape(BL, OUT, T).transpose(0, 2, 1)       # [BL, T, OUT]
        outs.append(o.astype(np.float32))
    out = np.concatenate(outs, axis=0)
    if np.any(b_out):
        out = out + b_out.astype(np.float32).reshape(1, 1, OUT)
    return out


# revision 9
# speedup vs baseline: 1.0297x; 1.0297x over previous
"""Trainium2 Bass kernel for nn_DiscreteTimeS4.

Model (reference):
    x_proj = relu(x_seq @ W_in^T + b_in)                  # [B, T, P]
    h_t = a * h_{t-1} + x_proj_t @ B ;  y_t = h_t @ C     # diagonal SSM scan
    out = y @ W_out^T + b_out                             # [B, T, OUT]

Key transform: |a| <= sqrt(2/H) ~ 0.09, so a^k decays below the 2e-2
correctness gate within a couple of steps.  The scan is a short causal
convolution over time with W_out folded in:
    out_t = sum_k x_proj_{t-k} @ F_k,   F_k = B @ diag(a^k) @ C @ W_out^T

Device pipeline (per core, batch rows processed in PAIRS so that the
full 128-partition dim is used with no duplicated input DMA):
    stage 1: for row pair rp, chunk c: two row-group-packed matmuls
             (K=64 contraction each) -> PSUM [128, 2*512] (both rows),
             then one fused bias+relu op (DVE or ACT) -> xp tile f16.
    stage 2: for chunk pair: per chunk, K taps x 2 col-groups of
             F_k^T @ xp(shift k), PSUM-accumulated -> [128(=2 rows x
             64 outs), 1024]; lag shifts are SBUF column offsets.
    out:     one copy op (DVE/ACT) PSUM -> SBUF f16, then one
             contiguous DMA -> out dram [BL*OUT, T] f16.
Host side: fold F_k in fp64, pre-transpose x to [BL*IN, T] f16,
post-transpose out back to [B, T, OUT] fp32 (b_out is zero here but
added on host if nonzero).

Sharding: data-parallel over batch, 8 NeuronCores, B=64 -> 8 per core.
"""

import os
import sys

for _p in ("/opt/trn_rl_repo", "/root/.axon_site/_ro/trn_rl_repo"):
    if os.path.isdir(_p) and _p not in sys.path:
        sys.path.append(_p)

import numpy as np

import concourse.bacc as bacc
import concourse.mybir as mybir
from concourse.bass_utils import run_bass_kernel_spmd
from concourse.tile import TileContext

BATCH, T, IN, P, H, OUT = 64, 2048, 64, 128, 256, 64
NCORES = 8
BL = BATCH // NCORES          # batches per core
RP = BL // 2                  # row pairs per core
CHUNK = 512                   # time chunk (one fp32 PSUM bank)
NCHUNK = T // CHUNK
NP2 = NCHUNK // 2             # chunk pairs per row

F32 = mybir.dt.float32
F16 = mybir.dt.float16

_programs = {}                # (n_lags, reps) -> finalized Bacc program


def _build(n_lags: int, reps: int = 1):
    """Build the per-core Bass program for a fixed lag count.

    reps > 1 wraps the whole computation in an on-device loop executing
    it `reps` times — used only for benchmarking (amortizes the axon
    dispatch overhead, which dwarfs the kernel itself).
    """
    import contextlib

    nc = bacc.Bacc("TRN2", target_bir_lowering=False, num_devices=NCORES)

    x = nc.declare_dram_parameter("x", [BL * IN, T], F16, isOutput=False)
    # fk taps (k-major) and the duplicated W_in^T packed in one tensor so
    # all f16 weights arrive in a single DMA
    wpack = nc.declare_dram_parameter("wpack", [P, n_lags * OUT + P], F16,
                                      isOutput=False)
    b_in = nc.declare_dram_parameter("b_in", [P, 1], F32, isOutput=False)
    out = nc.declare_dram_parameter("out", [BL * OUT, T], F16, isOutput=True)

    with TileContext(nc) as tc:
        with (
            tc.tile_pool(name="wpool", bufs=1) as wpool,
            tc.tile_pool(name="xin", bufs=2) as xin_pool,
            tc.tile_pool(name="xproj", bufs=2) as xp_pool,
            tc.tile_pool(name="obuf", bufs=2) as ob_pool,
            tc.tile_pool(name="ps1", bufs=2, space="PSUM") as ps1_pool,
            tc.tile_pool(name="pso", bufs=2, space="PSUM") as pso_pool,
        ):
            # ---- load weights once (already fp16 from host) ----
            wboth = wpool.tile([P, n_lags * OUT + P], F16)
            nc.sync.dma_start(out=wboth[:], in_=wpack[:])
            bi = wpool.tile([P, 1], F32)
            nc.sync.dma_start(out=bi[:], in_=b_in[:])

            rep_ctx = (
                tc.For_i(
                    0, reps, 1,
                    hint_engines=(
                        mybir.EngineType.PE,
                        mybir.EngineType.DVE,
                        mybir.EngineType.Activation,
                        mybir.EngineType.SP,
                        mybir.EngineType.Pool,
                    ),
                )
                if reps > 1
                else contextlib.nullcontext()
            )
            with rep_ctx:
                _emit_body(nc, tc, n_lags, x, out, wboth, bi,
                           xin_pool, xp_pool, ob_pool, ps1_pool, pso_pool)

    nc.finalize()
    return nc


def _emit_body(nc, tc, n_lags, x, out, wboth, bi,
               xin_pool, xp_pool, ob_pool, ps1_pool, pso_pool):
    PAD = n_lags - 1
    XROW = PAD + T              # x_proj columns per row half
    KO = n_lags * OUT           # wboth column where W_in^T starts

    # Elementwise engine balancing: DVE ~245 G elem/s, ACT ~154 G elem/s;
    # ACT is pre-loaded with two input-DMA issues (HWDGE on its queue).
    load = {"v": 0.0, "a": 3.4}
    COST = {"v": 0.74, "a": 1.15}

    def pick_engine():
        e = "v" if load["v"] + COST["v"] <= load["a"] + COST["a"] else "a"
        load[e] += COST[e]
        return e

    def load_x(rp):
        xTr = xin_pool.tile([2 * IN, T], F16, tag="xTr")
        eng = nc.sync if rp % 2 == 0 else nc.scalar
        eng.dma_start(out=xTr[:], in_=x[rp * 128:(rp + 1) * 128, :])
        return xTr

    def stage1(rp, c, xTr, xp_rp):
        """Input projection + bias-relu for chunk c of both rows of pair
        rp, written densely into the per-pair x_proj tile [128, 2*XROW]
        (halves = rows 2rp, 2rp+1; PAD zero cols at each half's front)."""
        ps1 = ps1_pool.tile([P, 2 * CHUNK], F32)
        for h in range(2):
            nc.tensor.matmul(
                ps1[:, h * CHUNK:(h + 1) * CHUNK],
                wboth[h * IN:(h + 1) * IN, KO:KO + P],
                xTr[h * IN:(h + 1) * IN, c * CHUNK:(c + 1) * CHUNK],
                start=True, stop=True,
                tile_position=(h * IN, 0),
            )
        xp3 = xp_rp[:].rearrange("p (h z) -> p h z", h=2)
        dst = xp3[:, :, PAD + c * CHUNK:PAD + (c + 1) * CHUNK]
        src = ps1[:].rearrange("p (h t) -> p h t", h=2)
        if pick_engine() == "v":
            nc.vector.tensor_scalar(
                out=dst, in0=src, scalar1=bi[:], scalar2=0.0,
                op0=mybir.AluOpType.add, op1=mybir.AluOpType.max,
            )
        else:
            nc.scalar.activation(
                out=dst, in_=src,
                func=mybir.ActivationFunctionType.Relu, bias=bi[:],
            )

    def stage2(rp, p, xp_rp, ob_rp):
        """Fused conv for chunk pair (2p, 2p+1) of row pair rp.
        Col-groups h = the two rows run concurrently in disjoint PE
        column groups -> pso [128 (= 2 rows x 64 outs), 1024]."""
        pso = pso_pool.tile([2 * OUT, 2 * CHUNK], F32)
        xp3 = xp_rp[:].rearrange("p (h z) -> p h z", h=2)
        for q in range(2):
            base = PAD + (2 * p + q) * CHUNK
            for k in range(n_lags):
                for h in range(2):
                    nc.tensor.matmul(
                        pso[h * OUT:(h + 1) * OUT,
                            q * CHUNK:(q + 1) * CHUNK],
                        wboth[:, k * OUT:(k + 1) * OUT],
                        xp3[:, h, base - k:base - k + CHUNK],
                        start=(k == 0), stop=(k == n_lags - 1),
                        tile_position=(0, h * OUT),
                    )
        if pick_engine() == "v":
            nc.vector.tensor_copy(
                out=ob_rp[:, p * 2 * CHUNK:(p + 1) * 2 * CHUNK], in_=pso[:])
        else:
            nc.scalar.activation(
                out=ob_rp[:, p * 2 * CHUNK:(p + 1) * 2 * CHUNK], in_=pso[:],
                func=mybir.ActivationFunctionType.Copy)
        if p == NP2 - 1:
            # whole row pair evacuated: one SWDGE store on the idle Pool
            # queue, parallel to the HWDGE input loads
            nc.gpsimd.dma_start(out=out[rp * 128:(rp + 1) * 128, :],
                                in_=ob_rp[:])

    # Software pipeline: stage2 of a chunk pair is emitted DEPTH
    # stage1-steps after its inputs are produced, so PE work stays dense
    # while bias-relu results land.
    DEPTH = 2
    s2q = []
    xTr_cur = None
    xp_cur = None
    ob = {}
    for rp in range(RP):
        for c in range(NCHUNK):
            if c == 0:
                xTr_cur = load_x(rp)
                xp_cur = xp_pool.tile([P, 2 * XROW], F16, tag="xpp")
                if PAD > 0:
                    xp3 = xp_cur[:].rearrange("p (h z) -> p h z", h=2)
                    nc.gpsimd.memset(xp3[:, :, 0:PAD], 0.0)
                ob_rp = ob_pool.tile([2 * OUT, T], F16, tag="ob")
                ob[rp] = ob_rp
            stage1(rp, c, xTr_cur, xp_cur)
            if c % 2 == 1:
                s2q.append((rp, c // 2, xp_cur, ob[rp]))
            while len(s2q) > DEPTH:
                stage2(*s2q.pop(0))
    while s2q:
        stage2(*s2q.pop(0))


def _n_lags(a: np.ndarray) -> int:
    amax = float(np.abs(a).max())
    if amax >= 1.0:
        return 16
    if amax <= 0.0:
        return 2
    # Correctness gate is rel_err < 2e-2 (max-normalized).  Truncating
    # at a^k < 8e-3 gives measured end-to-end error ~3.7e-3 for this
    # model (K=2), a 5x margin; the fp16 operand noise floor is ~5e-4.
    k = int(np.ceil(np.log(8e-3) / np.log(amax)))
    return max(2, min(16, k))


def _prepare(x_seq, a, B, C, W_in, b_in, W_out, b_out):
    """Host-side folding + per-core input maps."""
    n_lags = _n_lags(a)
    a64 = a.astype(np.float64)
    B64 = B.astype(np.float64)
    C64 = C.astype(np.float64)
    CW64 = C64 @ W_out.T.astype(np.float64)                # [H, OUT]
    fks = np.stack(
        [(B64 * (a64 ** k)[None, :]) @ CW64 for k in range(n_lags)]
    ).astype(np.float16)                                   # [K, P, OUT]
    wiT = W_in.T.astype(np.float16)
    wpack = np.concatenate(
        [fks.transpose(1, 0, 2).reshape(P, n_lags * OUT),  # k-major taps
         np.vstack([wiT, wiT])], axis=1)                   # [P, K*OUT + P]
    shared = {
        "wpack": np.ascontiguousarray(wpack),
        "b_in": np.ascontiguousarray(b_in.astype(np.float32).reshape(P, 1)),
    }
    xT = np.swapaxes(x_seq, 1, 2).astype(np.float16)       # [B, IN, T]
    in_maps = []
    for c in range(NCORES):
        m = dict(shared)
        m["x"] = np.ascontiguousarray(
            xT[c * BL:(c + 1) * BL].reshape(BL * IN, T))
        in_maps.append(m)
    return n_lags, in_maps


def get_program(n_lags: int, reps: int = 1):
    key = (n_lags, reps)
    if key not in _programs:
        _programs[key] = _build(n_lags, reps)
    return _programs[key]


def kernel(x_seq, a, B, C, W_in, b_in, W_out, b_out):
    n_lags, in_maps = _prepare(x_seq, a, B, C, W_in, b_in, W_out, b_out)
    nc = get_program(n_lags)
    res = run_bass_kernel_spmd(nc, in_maps, list(range(NCORES)))
    outs = []
    for c in range(NCORES):
        o = np.asarray(res.results[c]["out"])              # [BL*OUT, T] f16
        o = o.reshape(BL, OUT, T).transpose(0, 2, 1)       # [BL, T, OUT]
        outs.append(o.astype(np.float32))
    out = np.concatenate(outs, axis=0)
    if np.any(b_out):
        out = out + b_out.astype(np.float32).reshape(1, 1, OUT)
    return out
